# revision 1
# baseline (speedup 1.0000x reference)
"""Causal single-head attention (B=4, S=2048, D=1024) on 8 TRN2 NeuronCores.

Sharding: core c -> (batch b = c//2, half h = c%2). Every core runs the SAME
program: 8 query tiles of 128 rows whose padded causal key-lengths are
L_s = 256*(s+1) for s=0..7.  Core (b, h) takes global query rows
[256*s + 128*h, 256*s + 128*h + 128) of batch b for slot s.

All device operands are bf16 (quantized host-side; PSUM accumulates fp32).
X and W are shipped pre-transposed/reshaped as [P, DC, cols] so the
contraction dim d sits on the partitions for every projection matmul and the
device never transposes activations.

Phases: G^T -> attention; there is NO K or V projection on device.
Both sides are reassociated so every GEMM scales with this core's 1024
queries instead of the 2048 keys: scores = X_q (Wq Wk^T) X_k^T with the
weight-only product Wqk = Wq@Wk^T fused host-side (G^T = Wqk^T X_q^T is the
only projection phase), and out = ((attn @ X_v) @ W_v) / den (U^T).  The
per-core-pair duplicated K and V projections disappear entirely; X_k^T and
X_v are simply loaded resident.  Weights are prefetched one phase ahead, X streamed in 512-column blocks,
and the first K block runs dc-outer so compute starts as soon as the first
weight/X chunks land.  Attention computes scores TRANSPOSED (scores^T[k, q],
keys on partitions): exp output lands directly in the attn^T layout that both
the U^T matmul (rhs) and nothing else needs -- no PE transposes anywhere
(U^T is produced directly with the natural-layout X_v chunk as stationary).
The causal mask is two data-driven [128, 128] tiles (h-dependent) added to
the last two key tiles of each slot; the softmax denominator is an ap=1
matmul against a ones vector; 1/den is applied on the Y = U @ W_v copy-out
(ACT low half, DVE tensor_scalar high half, in parallel).  Slots run
longest-first and are software-pipelined at score-group granularity so the
PE never waits on an exp.

NOTE (hazard found empirically): interleaving start/stop matmul accumulation
chains across sub-regions of a single PSUM bank corrupts results on HW;
interleaving across distinct banks is fine.  Keep each sub-tile's ec-loop
contiguous.
"""

import numpy as np

import concourse.bacc as bacc
import concourse.mybir as mybir
import concourse.tile as tile
from concourse import bass_utils

B, S, D = 4, 2048, 1024
P = 128
DC = D // P          # 8 contraction chunks
EC = D // P          # 8 output-dim chunks
NSLOT = 8            # q tiles per core
NQ = NSLOT * P       # 1024 q rows per core
SCALE = 1.0 / float(np.sqrt(np.float32(S)))
NEG = -1.0e9

F32 = mybir.dt.float32
BF16 = mybir.dt.bfloat16


def build_attention_nc():
    nc = bacc.Bacc("TRN2", target_bir_lowering=False)

    xqT = nc.dram_tensor("xqT", [P, DC, NQ], BF16, kind="ExternalInput")
    xkT = nc.dram_tensor("xkT", [P, DC, S], BF16, kind="ExternalInput")
    xv_in = nc.dram_tensor("xv", [P, S // P, D], BF16, kind="ExternalInput")
    wqk = nc.dram_tensor("wqk", [P, DC, D], BF16, kind="ExternalInput")
    wv = nc.dram_tensor("wv", [P, DC, D], BF16, kind="ExternalInput")
    mask_a_in = nc.dram_tensor("mask_a", [P, P], BF16, kind="ExternalInput")
    mask_b_in = nc.dram_tensor("mask_b", [P, P], BF16, kind="ExternalInput")
    ones_in = nc.dram_tensor("ones", [P, 1], BF16, kind="ExternalInput")
    out = nc.dram_tensor("out", [NQ, D], BF16, kind="ExternalOutput")

    with tile.TileContext(nc) as tc:
        with (
            tc.tile_pool(name="res", bufs=1) as res,
            tc.tile_pool(name="psum", bufs=4, space="PSUM") as pp,
        ):
            kt_sb = res.tile([P, EC, S], BF16)      # K^T  [e, keys]
            xv_sb = res.tile([P, S // P, D], BF16)  # X_v  [keys, d] (natural)
            qt_sb = res.tile([P, EC, NQ], BF16)     # Q^T  [e, q]
            wv_rs = res.tile([P, DC, D], BF16)      # W_v resident for U@Wv
            mask_a = res.tile([P, P], BF16)
            mask_b = res.tile([P, P], BF16)
            ones_sb = res.tile([P, 1], BF16)

            # ============ projection phases (K^T, V, Q^T) ============
            # One PSUM pool for the whole kernel: projection tiles and the
            # attention score tiles share tag "ps" ([P, 512]), so there is no
            # pool-close drain between phases.
            with (
                tc.tile_pool(name="wp", bufs=2) as wp,
                tc.tile_pool(name="xp", bufs=3) as xp,
            ):
                def load_w(w_t, chunks=2, engs=(nc.gpsimd,)):
                    w_sb = wp.tile([P, DC, D], BF16, tag="w")
                    n = DC // chunks
                    for i in range(chunks):
                        engs[i % len(engs)].dma_start(
                            w_sb[:, i * n : (i + 1) * n],
                            w_t[:, i * n : (i + 1) * n],
                        )
                    return w_sb

                def load_x(x_t, c0, width, split=2, engs=(nc.sync,)):
                    xb = xp.tile([P, DC, width], BF16, tag="x")
                    n = DC // split
                    for i in range(split):
                        engs[i % len(engs)].dma_start(
                            xb[:, i * n : (i + 1) * n],
                            x_t[:, i * n : (i + 1) * n, c0 : c0 + width],
                        )
                    return xb

                # Startup: interleave the critical first loads across the
                # shared HWDGE (sync+scalar) and the SWDGE (gpsimd) domains so
                # the first matmul can issue ~3us in.  The G^T qb=1 pass runs
                # dc-outer so compute starts as soon as (wqk[0], xq[0]) land.
                wqk_sb = wp.tile([P, DC, D], BF16, tag="w")
                xb0 = xp.tile([P, DC, 512], BF16, tag="x")
                nc.sync.dma_start(wqk_sb[:, 0, 0:512], wqk[:, 0, 0:512])
                nc.scalar.dma_start(xb0[:, 0], xqT[:, 0, 512:NQ])
                nc.sync.dma_start(wqk_sb[:, 0, 512:D], wqk[:, 0, 512:D])
                for dc in range(1, DC):
                    nc.gpsimd.dma_start(wqk_sb[:, dc], wqk[:, dc])
                for dc in range(1, DC):
                    eng = nc.sync if dc % 2 else nc.scalar
                    eng.dma_start(xb0[:, dc], xqT[:, dc, 512:NQ])
                xb_q0 = load_x(xqT, 0, 512, split=4, engs=(nc.sync, nc.scalar))
                nc.gpsimd.dma_start(kt_sb[:, :, 0:512], xkT[:, :, 0:512])
                nc.gpsimd.dma_start(kt_sb[:, :, 512:1024], xkT[:, :, 512:1024])
                nc.gpsimd.dma_start(wv_rs[:, 0:4], wv[:, 0:4])
                nc.gpsimd.dma_start(wv_rs[:, 4:8], wv[:, 4:8])
                nc.gpsimd.dma_start(mask_a, mask_a_in[:, :])
                nc.gpsimd.dma_start(mask_b, mask_b_in[:, :])
                nc.gpsimd.dma_start(ones_sb, ones_in[:, :])
                for i in range(4):
                    nc.gpsimd.dma_start(
                        xv_sb[:, i * 4 : (i + 1) * 4, :],
                        xv_in[:, i * 4 : (i + 1) * 4, :],
                    )


                # ---- G^T phase: G = X_q @ (Wq Wk^T); qb=1 first so
                # attention slot 7 can start right after it ----
                for eh in range(2):
                    ps_list = [pp.tile([P, 512], F32, tag="ps", name=f"psk{eh}_{i}") for i in range(4)]
                    for dc in range(DC):
                        for i in range(4):
                            ec = eh * 4 + i
                            nc.tensor.matmul(
                                ps_list[i],
                                wqk_sb[:, dc, ec * P : (ec + 1) * P],
                                xb0[:, dc],
                                start=(dc == 0),
                                stop=(dc == DC - 1),
                            )
                    for i in range(4):
                        ec = eh * 4 + i
                        if i % 2 == 0:
                            nc.vector.tensor_copy(
                                qt_sb[:, ec, 512:NQ], ps_list[i]
                            )
                        else:
                            nc.scalar.copy(qt_sb[:, ec, 512:NQ], ps_list[i])
                xb = xb_q0
                # X_k^T resident (raw keys -- no K projection exists),
                # queued behind the qb=0 queries; X_v streams on SWDGE.
                for i in range(2, 4):
                    eng = nc.sync if i % 2 else nc.scalar
                    eng.dma_start(
                        kt_sb[:, :, i * 512 : (i + 1) * 512],
                        xkT[:, :, i * 512 : (i + 1) * 512],
                    )
                for ec in range(EC):
                    ps = pp.tile([P, 512], F32, tag="ps")
                    for dc in range(DC):
                        nc.tensor.matmul(
                            ps,
                            wqk_sb[:, dc, ec * P : (ec + 1) * P],
                            xb[:, dc],
                            start=(dc == 0),
                            stop=(dc == DC - 1),
                        )
                    if ec % 2 == 0:
                        nc.vector.tensor_copy(qt_sb[:, ec, 0:512], ps)
                    else:
                        nc.scalar.copy(qt_sb[:, ec, 0:512], ps)

            # ================= attention phase =================
            # Scores are computed TRANSPOSED (scores^T[k, q], keys on
            # partitions): exp then lands directly in the attn^T layout the
            # attn@V matmul wants as stationary -- no PE transposes at all.
            # The softmax denominator is an ap=1 matmul against a ones vector
            # accumulated over key tiles (essentially free on the PE).
            with tc.tile_pool(name="attn", bufs=2) as attnp:
                # Software pipeline at group granularity: the den/attn@V
                # matmuls of group (s, g) are emitted after the NEXT group's
                # scores+exp, so the PE never sits waiting on an exp -- there
                # is always a ready matmul in program order.
                slot_state = {}
                pending = []

                def consume(s, g):
                    st = slot_state[s]
                    nt = st["nt"]
                    cnt = min(4, nt - g * 4)
                    for i in range(cnt):
                        t = g * 4 + i
                        nc.tensor.matmul(
                            st["ps_den"],
                            st["attnT"][:, t, :],
                            ones_sb,
                            start=(t == 0),
                            stop=(t == nt - 1),
                        )
                    if g == st["ng"] - 1:
                        # slot finished: U^T = Xv^T @ attn^T directly (d on
                        # partitions, Xv natural layout as stationary), then
                        # Y = U @ Wv with 1/den applied on copy-out.
                        rec = attnp.tile([P, 1], F32, tag="rec")
                        nc.vector.reciprocal(rec, st["ps_den"])
                        ut = attnp.tile([P, DC, P], BF16, tag="ut", name=f"ut{s}")
                        for dc in range(DC):
                            ps_u = pp.tile(
                                [P, P], F32, tag="utacc", bufs=3,
                                name=f"psu{s}_{dc}",
                            )
                            for t in range(nt):
                                nc.tensor.matmul(
                                    ps_u,
                                    xv_sb[:, t, dc * P : (dc + 1) * P],
                                    st["attnT"][:, t, :],
                                    start=(t == 0),
                                    stop=(t == nt - 1),
                                )
                            nc.vector.tensor_copy(ut[:, dc, :], ps_u)
                        out_sb = attnp.tile([P, D], BF16, tag="out", bufs=3)
                        for eh in range(2):
                            ps_y = pp.tile(
                                [P, 512], F32, tag="ps", bufs=4,
                                name=f"psy{eh}_{s}",
                            )
                            for dc in range(DC):
                                nc.tensor.matmul(
                                    ps_y,
                                    ut[:, dc, :],
                                    wv_rs[:, dc, eh * 512 : (eh + 1) * 512],
                                    start=(dc == 0),
                                    stop=(dc == DC - 1),
                                )
                            if eh == 0 and s == 0:
                                nc.scalar.activation(
                                    out=out_sb[:, 0:512],
                                    in_=ps_y,
                                    func=mybir.ActivationFunctionType.Copy,
                                    scale=rec,
                                )
                            else:
                                nc.vector.tensor_scalar_mul(
                                    out_sb[:, eh * 512 : (eh + 1) * 512],
                                    ps_y,
                                    rec,
                                )
                            nc.sync.dma_start(
                                out[s * P : (s + 1) * P, eh * 512 : (eh + 1) * 512],
                                out_sb[:, eh * 512 : (eh + 1) * 512],
                            )
                        del slot_state[s]

                for s in range(NSLOT - 1, -1, -1):
                    L = 256 * (s + 1)
                    nt = L // P
                    ng = (nt + 3) // 4
                    slot_state[s] = {
                        "nt": nt,
                        "ng": ng,
                        "attnT": attnp.tile(
                            [P, S // P, P], BF16, tag="attnT", bufs=3,
                            name=f"attnT{s}",
                        ),
                        "ps_den": pp.tile(
                            [P, 1], F32, tag="ps_den", bufs=1, name=f"psden{s}"
                        ),
                    }
                    for g in range(ng):
                        cnt = min(4, nt - g * 4)
                        psT = pp.tile([P, 512], F32, tag="ps")
                        for i in range(cnt):
                            t = g * 4 + i
                            for ec in range(EC):
                                nc.tensor.matmul(
                                    psT[:, i * P : (i + 1) * P],
                                    kt_sb[:, ec, t * P : (t + 1) * P],
                                    qt_sb[:, ec, s * P : (s + 1) * P],
                                    start=(ec == 0),
                                    stop=(ec == EC - 1),
                                )
                        if g == ng - 1:
                            # causal mask on the last two key tiles: for h=0
                            # mask_a is the triangle and mask_b is all -1e9;
                            # for h=1 mask_a is zero and mask_b the triangle.
                            nc.vector.tensor_add(
                                out=psT[:, (cnt - 2) * P : (cnt - 1) * P],
                                in0=psT[:, (cnt - 2) * P : (cnt - 1) * P],
                                in1=mask_a,
                            )
                            nc.vector.tensor_add(
                                out=psT[:, (cnt - 1) * P : cnt * P],
                                in0=psT[:, (cnt - 1) * P : cnt * P],
                                in1=mask_b,
                            )
                        nc.scalar.activation(
                            out=slot_state[s]["attnT"][:, g * 4 : g * 4 + cnt, :],
                            in_=psT[:, : cnt * P],
                            func=mybir.ActivationFunctionType.Exp,
                            scale=SCALE,
                        )
                        if pending:
                            consume(*pending.pop(0))
                        pending.append((s, g))
                while pending:
                    consume(*pending.pop(0))

    nc.compile()
    return nc


_NC_CACHE = None


def _get_nc():
    global _NC_CACHE
    if _NC_CACHE is None:
        _NC_CACHE = build_attention_nc()
    return _NC_CACHE


def _make_masks(h: int) -> tuple[np.ndarray, np.ndarray]:
    """Transposed masks [key kk, query r] for the last two key tiles."""
    import ml_dtypes

    kk = np.arange(P)[:, None]
    r = np.arange(P)[None, :]
    tri = np.where(kk <= r, 0.0, NEG).astype(np.float32)
    if h == 0:
        mask_a, mask_b = tri, np.full((P, P), NEG, dtype=np.float32)
    else:
        mask_a, mask_b = np.zeros((P, P), dtype=np.float32), tri
    return mask_a.astype(ml_dtypes.bfloat16), mask_b.astype(ml_dtypes.bfloat16)


def kernel(
    inputs_for_keys,
    inputs_for_values,
    inputs_for_queries,
    weight_K,
    weight_V,
    weight_Q,
    trace=False,
):
    import ml_dtypes

    bf16 = ml_dtypes.bfloat16

    def _xT(x):  # [rows, D] f32 -> [P, DC, rows] bf16 (transposed, p-major)
        xt = np.asarray(x, dtype=np.float32).T.reshape(DC, P, x.shape[0])
        return np.ascontiguousarray(xt.transpose(1, 0, 2)).astype(bf16)

    def _w(w):  # [D, D] f32 -> [P, DC, D] bf16 (d_in on partitions, p-major)
        wr = np.asarray(w, dtype=np.float32).reshape(DC, P, D)
        return np.ascontiguousarray(wr.transpose(1, 0, 2)).astype(bf16)

    xk_full = np.asarray(inputs_for_keys, dtype=np.float32)
    xv_full = np.asarray(inputs_for_values, dtype=np.float32)
    xq_full = np.asarray(inputs_for_queries, dtype=np.float32)

    w_v = _w(weight_V)
    w_qk = _w(
        np.asarray(weight_Q, dtype=np.float32)
        @ np.asarray(weight_K, dtype=np.float32).T
    )

    def _xv(x):  # [S, D] f32 -> [P, S//P, D] bf16 (keys on partitions)
        xr = np.asarray(x, dtype=np.float32).reshape(S // P, P, D)
        return np.ascontiguousarray(xr.transpose(1, 0, 2)).astype(bf16)

    xkT = [_xT(xk_full[b]) for b in range(B)]
    xv = [_xv(xv_full[b]) for b in range(B)]

    masks = [_make_masks(0), _make_masks(1)]
    ones_np = np.ones((P, 1), dtype=np.float32).astype(bf16)
    in_maps = []
    for c in range(2 * B):
        b, h = c // 2, c % 2
        rows = np.concatenate(
            [
                xq_full[b, 256 * s + 128 * h : 256 * s + 128 * h + P, :]
                for s in range(NSLOT)
            ],
            axis=0,
        )
        in_maps.append(
            {
                "xqT": _xT(rows),
                "xkT": xkT[b],
                "xv": xv[b],
                "wqk": w_qk,
                "wv": w_v,
                "mask_a": masks[h][0],
                "mask_b": masks[h][1],
                "ones": ones_np,
            }
        )

    nc = _get_nc()
    res = bass_utils.run_bass_kernel_spmd(
        nc, in_maps, core_ids=list(range(2 * B)), trace=trace
    )

    out = np.empty((B, S, D), dtype=np.float32)
    for c in range(2 * B):
        b, h = c // 2, c % 2
        o = np.asarray(res.results[c]["out"], dtype=np.float32)
        for s in range(NSLOT):
            out[b, 256 * s + 128 * h : 256 * s + 128 * h + P, :] = o[
                s * P : (s + 1) * P, :
            ]

    if trace:
        return out, res
    return out



# revision 2
# speedup vs baseline: 1.4187x; 1.4187x over previous
"""Causal single-head attention (B=4, S=2048, D=1024) on 8 TRN2 NeuronCores.

fp8(e4m3) DoubleRow rewrite of the bf16 baseline.  Same sharding: core
c -> (batch b = c//2, half h = c%2); 8 query slots of 128 rows with padded
causal key-lengths L_s = 256*(s+1); scores computed transposed; Wqk = Wq@Wk^T
fused host-side; out = ((attn @ X_v) @ W_v) / den.

Quantization scheme (all matmuls fp8e4 DoubleRow, PSUM fp32):
  G = Xq @ Wqk      2-product: Wqk shipped as exact hi+lo fp8 pair (host),
                    Xq single fp8.  G -> qt hi+lo on device (ACT copy + DVE
                    scalar_tensor_tensor residual).
  scores = qt@K^T   2-product: qt hi+lo (device), K^T single fp8 (host).
  U = attn @ Xv     3-product: attn hi+lo (ACT exp f8 + bf16, DVE sub),
                    Xv hi+lo (host).  U -> ut hi+lo on device.
  Y = U @ Wv        3-product: ut hi+lo (device), Wv hi+lo (host).
hi+lo fp8 pairs represent bf16 values exactly, so the residual GEMMs have
~bf16-level error; the 2-product GEMMs err only by the single side's fp8
quantization.  The softmax denominator is summed from the quantized bf16
attn (== hi+lo exactly), cancelling common-mode quantization error; the
"ones" den vector carries value au*aw so 1/den absorbs all output scales.
Scales are powers of two shipped in a [P,4] f32 vector, so the compiled
program is input-independent.

Schedule: the cost model serializes all DMA on one shared resource
(~360 GB/s) with ~0.6us per-issue overhead, so all bulk input DMA goes on
ONE queue (sync) in exact first-consumption order; consts + Xq ride the
scalar queue; output DMA on the SWDGE.  Slots run ASCENDING (0..7) so K/V
chunks stream just-in-time, and the 8 Y GEMMs are deferred to the end
(ut hi/lo tiles are tiny and stay resident) where they keep the PE busy
through the tail while Wv has long arrived.

NOTE (hazard, empirical): interleaving start/stop matmul accumulation chains
across sub-regions of a single PSUM bank corrupts results on HW; keep each
sub-tile's chain contiguous (interleaving across banks is fine).
"""

import numpy as np

import concourse.bacc as bacc
import concourse.mybir as mybir
import concourse.tile as tile
from concourse import bass_utils

B, S, D = 4, 2048, 1024
P = 128
DC = D // P          # 8 contraction chunks
DCP = DC // 2        # 4 DoubleRow contraction pairs
EC = D // P
NSLOT = 8
NQ = NSLOT * P
SCALE = 1.0 / float(np.sqrt(np.float32(S)))
NEG = -1.0e9

F32 = mybir.dt.float32
BF16 = mybir.dt.bfloat16
F8 = mybir.dt.float8e4
DR = mybir.MatmulPerfMode.DoubleRow
MULT = mybir.AluOpType.mult
SUB = mybir.AluOpType.subtract


def build_attention_nc():
    nc = bacc.Bacc("TRN2", target_bir_lowering=False)

    xq_in = nc.dram_tensor("xq", [P, DC, NQ], F8, kind="ExternalInput")
    xk_in = nc.dram_tensor("xk", [P, EC, S], F8, kind="ExternalInput")
    # X_v hi/lo interleaved per key tile: [p, tile, {hi,lo}, d]
    xv_in = nc.dram_tensor("xv", [P, S // P, 2, D], F8, kind="ExternalInput")
    # Wqk packed in consumption-ordered chunks: chunk c = (colhalf a, dcpair
    # dp) at c = a*4+dp, holding [{hi,lo}, {row0,row1}, 512 cols]
    wq_in = nc.dram_tensor("wq", [P, 8, 2, 2, 512], F8, kind="ExternalInput")
    wvh_in = nc.dram_tensor("wvh", [P, DC, D], F8, kind="ExternalInput")
    wvl_in = nc.dram_tensor("wvl", [P, DC, D], F8, kind="ExternalInput")
    mask_a_in = nc.dram_tensor("mask_a", [P, P], BF16, kind="ExternalInput")
    mask_b_in = nc.dram_tensor("mask_b", [P, P], BF16, kind="ExternalInput")
    ones_in = nc.dram_tensor("ones", [P, 1], BF16, kind="ExternalInput")
    scl_in = nc.dram_tensor("scl", [P, 4], F32, kind="ExternalInput")
    out = nc.dram_tensor("out", [NQ, D], BF16, kind="ExternalOutput")

    with tile.TileContext(nc) as tc:
        with (
            tc.tile_pool(name="res", bufs=1) as res,
            tc.tile_pool(name="psum", bufs=4, space="PSUM") as pp,
        ):
            kt_sb = res.tile([P, EC, S], F8)        # K^T  [e, keys]
            xv_sb = res.tile([P, S // P, 2, D], F8)  # X_v hi/lo interleaved
            xq_sb = res.tile([P, DC, NQ], F8)       # X_q^T [d, q]
            qth_sb = res.tile([P, EC, NQ], F8)      # Q^T hi [e, q]
            qtl_sb = res.tile([P, EC, NQ], F8)      # Q^T lo
            wq_sb = res.tile([P, 8, 2, 2, 512], F8)  # Wqk packed chunks
            wvh_sb = res.tile([P, DC, D], F8)
            wvl_sb = res.tile([P, DC, D], F8)
            uth_sb = res.tile([P, NSLOT, DC, P], F8)  # U^T hi (all slots)
            utl_sb = res.tile([P, NSLOT, DC, P], F8)
            rec_sb = res.tile([P, NSLOT], F32)        # 1/(c*den) per slot
            mask_a = res.tile([P, P], BF16)
            mask_b = res.tile([P, P], BF16)
            ones_sb = res.tile([P, 1], BF16)
            scl_sb = res.tile([P, 4], F32)
            s_exp = scl_sb[:, 0:1]   # exp scale = SCALE/(ag*ak)
            s_qt = scl_sb[:, 1:2]    # gamma_g: psum->qt scale
            s_ut = scl_sb[:, 2:3]    # gamma_u: psum->ut scale

            # ---- DMA: Xq-qb0 + consts on scalar; all bulk on sync, in
            # first-consumption order.  Each issue costs ~0.63us on its
            # queue, so the stream uses few, large chunks. ----
            nc.scalar.dma_start(xq_sb[:, :, 0:512], xq_in[:, :, 0:512])
            nc.scalar.dma_start(scl_sb, scl_in[:, :])
            nc.scalar.dma_start(ones_sb, ones_in[:, :])
            nc.scalar.dma_start(mask_a, mask_a_in[:, :])
            nc.scalar.dma_start(mask_b, mask_b_in[:, :])
            nc.sync.dma_start(wq_sb[:, 0:1], wq_in[:, 0:1])
            nc.sync.dma_start(wq_sb[:, 1:4], wq_in[:, 1:4])
            nc.sync.dma_start(wq_sb[:, 4:6], wq_in[:, 4:6])
            nc.sync.dma_start(wq_sb[:, 6:8], wq_in[:, 6:8])
            nc.sync.dma_start(kt_sb[:, :, 0:512], xk_in[:, :, 0:512])
            nc.sync.dma_start(xv_sb[:, 0:2], xv_in[:, 0:2])
            nc.sync.dma_start(kt_sb[:, :, 512:1024], xk_in[:, :, 512:1024])
            nc.sync.dma_start(xv_sb[:, 2:4], xv_in[:, 2:4])
            nc.sync.dma_start(xq_sb[:, :, 512:NQ], xq_in[:, :, 512:NQ])
            nc.sync.dma_start(kt_sb[:, :, 1024:1536], xk_in[:, :, 1024:1536])
            nc.sync.dma_start(xv_sb[:, 4:8], xv_in[:, 4:8])
            nc.sync.dma_start(kt_sb[:, :, 1536:2048], xk_in[:, :, 1536:2048])
            nc.sync.dma_start(xv_sb[:, 8:12], xv_in[:, 8:12])
            nc.sync.dma_start(xv_sb[:, 12:16], xv_in[:, 12:16])
            nc.sync.dma_start(wvh_sb[:, :], wvh_in[:, :])
            nc.sync.dma_start(wvl_sb[:, :], wvl_in[:, :])

            def qt_copy(ec, cols, ps):
                nc.scalar.activation(
                    out=qth_sb[:, ec, cols], in_=ps,
                    func=mybir.ActivationFunctionType.Copy, scale=s_qt,
                )
                nc.vector.scalar_tensor_tensor(
                    out=qtl_sb[:, ec, cols], in0=ps, scalar=s_qt,
                    in1=qth_sb[:, ec, cols], op0=MULT, op1=SUB,
                )

            # ============ G^T phase ============
            # qb=0 (q cols 0:512 = slots 0-3, consumed first) dc-pair-outer
            # so compute starts as soon as the first wqk/xq chunks land.
            # wq_sb chunk c = (colhalf a)*4 + dp holds [{hi,lo}, pair, 512].
            for a in range(2):  # column half == ec group
                ps_list = [
                    pp.tile([P, 512], F32, tag="ps", name=f"psg{a}_{i}")
                    for i in range(4)
                ]
                for dp in range(DCP):
                    for i in range(4):
                        co = slice(i * P, (i + 1) * P)
                        nc.tensor.matmul(
                            ps_list[i], wq_sb[:, a * 4 + dp, 0, :, co],
                            xq_sb[:, 2 * dp:2 * dp + 2, 0:512],
                            start=(dp == 0), stop=False, perf_mode=DR,
                        )
                        nc.tensor.matmul(
                            ps_list[i], wq_sb[:, a * 4 + dp, 1, :, co],
                            xq_sb[:, 2 * dp:2 * dp + 2, 0:512],
                            start=False, stop=(dp == DCP - 1),
                            perf_mode=DR,
                        )
                for i in range(4):
                    qt_copy(a * 4 + i, slice(0, 512), ps_list[i])
            # qb=1 (q cols 512:1024) ec-outer; emitted lazily after slot 3
            # so the early slots aren't gated on the qb=1 Xq DMA
            def emit_gqb1():
                for ec in range(EC):
                    ps = pp.tile([P, 512], F32, tag="ps")
                    a, i = ec // 4, ec % 4
                    co = slice(i * P, (i + 1) * P)
                    for dp in range(DCP):
                        nc.tensor.matmul(
                            ps, wq_sb[:, a * 4 + dp, 0, :, co],
                            xq_sb[:, 2 * dp:2 * dp + 2, 512:NQ],
                            start=(dp == 0), stop=False, perf_mode=DR,
                        )
                        nc.tensor.matmul(
                            ps, wq_sb[:, a * 4 + dp, 1, :, co],
                            xq_sb[:, 2 * dp:2 * dp + 2, 512:NQ],
                            start=False, stop=(dp == DCP - 1), perf_mode=DR,
                        )
                    qt_copy(ec, slice(512, NQ), ps)

            # ================= attention =================
            with tc.tile_pool(name="attn", bufs=2) as ap:
                slot_state = {}
                pending = []

                def consume(s, g):
                    st = slot_state[s]
                    nt = st["nt"]
                    cnt = min(4, nt - g * 4)
                    a16, ah, al = st["a16"], st["ah"], st["al"]
                    for i in range(cnt):
                        t = g * 4 + i
                        nc.tensor.matmul(
                            st["ps_den"], a16[:, t, :], ones_sb,
                            start=(t == 0), stop=(t == nt - 1),
                        )
                    if g != st["ng"] - 1:
                        return
                    # slot finished: U^T hi/lo; Y deferred to the end
                    nc.vector.reciprocal(
                        rec_sb[:, s:s + 1], st["ps_den"]
                    )
                    np_ = nt // 2
                    for dh in range(2):
                        ps_u = pp.tile(
                            [P, 512], F32, tag="ua", bufs=2,
                            name=f"psu{s}_{dh}",
                        )
                        for i in range(4):
                            dc = dh * 4 + i
                            ds = slice(dc * P, (dc + 1) * P)
                            po = slice(i * P, (i + 1) * P)
                            for kp in range(np_):
                                ks = slice(2 * kp, 2 * kp + 2)
                                nc.tensor.matmul(
                                    ps_u[:, po], xv_sb[:, ks, 0, ds],
                                    ah[:, ks, :],
                                    start=(kp == 0), stop=False, perf_mode=DR,
                                )
                                nc.tensor.matmul(
                                    ps_u[:, po], xv_sb[:, ks, 0, ds],
                                    al[:, ks, :],
                                    start=False, stop=False, perf_mode=DR,
                                )
                                nc.tensor.matmul(
                                    ps_u[:, po], xv_sb[:, ks, 1, ds],
                                    ah[:, ks, :],
                                    start=False, stop=(kp == np_ - 1),
                                    perf_mode=DR,
                                )
                        ucols = slice(dh * 4, dh * 4 + 4)
                        nc.scalar.activation(
                            out=uth_sb[:, s, ucols, :], in_=ps_u,
                            func=mybir.ActivationFunctionType.Copy,
                            scale=s_ut,
                        )
                        nc.vector.scalar_tensor_tensor(
                            out=utl_sb[:, s, ucols, :], in0=ps_u,
                            scalar=s_ut, in1=uth_sb[:, s, ucols, :],
                            op0=MULT, op1=SUB,
                        )
                    del slot_state[s]

                for s in range(NSLOT):
                    if s == 4:
                        emit_gqb1()
                        while pending:
                            consume(*pending.pop(0))
                    L = 256 * (s + 1)
                    nt = L // P
                    ng = (nt + 3) // 4
                    slot_state[s] = {
                        "nt": nt,
                        "ng": ng,
                        "a16": ap.tile(
                            [P, S // P, P], BF16, tag="a16", bufs=2,
                            name=f"a16_{s}",
                        ),
                        "ah": ap.tile(
                            [P, S // P, P], F8, tag="ah", bufs=2,
                            name=f"ah_{s}",
                        ),
                        "al": ap.tile(
                            [P, S // P, P], F8, tag="al", bufs=2,
                            name=f"al_{s}",
                        ),
                        "ps_den": pp.tile(
                            [P, 1], F32, tag="psden", bufs=2, name=f"psden{s}"
                        ),
                    }
                    st = slot_state[s]
                    qs = slice(s * P, (s + 1) * P)
                    for g in range(ng):
                        cnt = min(4, nt - g * 4)
                        psT = pp.tile([P, 512], F32, tag="ps")
                        for i in range(cnt):
                            t = g * 4 + i
                            po = slice(i * P, (i + 1) * P)
                            ts = slice(t * P, (t + 1) * P)
                            for j in range(DCP):
                                sl = slice(2 * j, 2 * j + 2)
                                nc.tensor.matmul(
                                    psT[:, po], kt_sb[:, sl, ts],
                                    qth_sb[:, sl, qs],
                                    start=(j == 0), stop=False, perf_mode=DR,
                                )
                                nc.tensor.matmul(
                                    psT[:, po], kt_sb[:, sl, ts],
                                    qtl_sb[:, sl, qs],
                                    start=False, stop=(j == DCP - 1),
                                    perf_mode=DR,
                                )
                        if g == ng - 1:
                            nc.vector.tensor_add(
                                out=psT[:, (cnt - 2) * P:(cnt - 1) * P],
                                in0=psT[:, (cnt - 2) * P:(cnt - 1) * P],
                                in1=mask_a,
                            )
                            nc.vector.tensor_add(
                                out=psT[:, (cnt - 1) * P:cnt * P],
                                in0=psT[:, (cnt - 1) * P:cnt * P],
                                in1=mask_b,
                            )
                        gs = slice(g * 4, g * 4 + cnt)
                        nc.scalar.activation(
                            out=st["a16"][:, gs, :], in_=psT[:, :cnt * P],
                            func=mybir.ActivationFunctionType.Exp,
                            scale=s_exp,
                        )
                        nc.scalar.activation(
                            out=st["ah"][:, gs, :], in_=psT[:, :cnt * P],
                            func=mybir.ActivationFunctionType.Exp,
                            scale=s_exp,
                        )
                        nc.vector.tensor_sub(
                            out=st["al"][:, gs, :],
                            in0=st["a16"][:, gs, :],
                            in1=st["ah"][:, gs, :],
                        )
                        if len(pending) >= 2:
                            consume(*pending.pop(0))
                        pending.append((s, g))
                while pending:
                    consume(*pending.pop(0))

                # ---- deferred Y = U @ Wv for all slots ----
                for s in range(NSLOT):
                    out_sb = ap.tile([P, D], BF16, tag="out", bufs=3)
                    rec = rec_sb[:, s:s + 1]
                    for eh in range(2):
                        es = slice(eh * 512, (eh + 1) * 512)
                        ps_y = pp.tile(
                            [P, 512], F32, tag="ps", bufs=4,
                            name=f"psy{eh}_{s}",
                        )
                        for dp in range(DCP):
                            sl = slice(2 * dp, 2 * dp + 2)
                            nc.tensor.matmul(
                                ps_y, uth_sb[:, s, sl, :], wvh_sb[:, sl, es],
                                start=(dp == 0), stop=False, perf_mode=DR,
                            )
                            nc.tensor.matmul(
                                ps_y, uth_sb[:, s, sl, :], wvl_sb[:, sl, es],
                                start=False, stop=False, perf_mode=DR,
                            )
                            nc.tensor.matmul(
                                ps_y, utl_sb[:, s, sl, :], wvh_sb[:, sl, es],
                                start=False, stop=(dp == DCP - 1),
                                perf_mode=DR,
                            )
                        for q in range(2):  # quarter-split for a short tail
                            qs_ = slice(eh * 512 + q * 256,
                                        eh * 512 + (q + 1) * 256)
                            if q == 0:
                                nc.scalar.activation(
                                    out=out_sb[:, qs_], in_=ps_y[:, 0:256],
                                    func=mybir.ActivationFunctionType.Copy,
                                    scale=rec,
                                )
                            else:
                                nc.vector.tensor_scalar_mul(
                                    out_sb[:, qs_], ps_y[:, 256:512], rec
                                )
                            eng = nc.gpsimd if (2 * eh + q) % 2 == 0 \
                                else nc.sync
                            eng.dma_start(
                                out[s * P:(s + 1) * P, qs_], out_sb[:, qs_]
                            )

    nc.compile()
    return nc


_NC_CACHE = None


def _get_nc():
    global _NC_CACHE
    if _NC_CACHE is None:
        _NC_CACHE = build_attention_nc()
    return _NC_CACHE


def _make_masks(h):
    """Transposed masks [key kk, query r] for the last two key tiles."""
    import ml_dtypes

    kk = np.arange(P)[:, None]
    r = np.arange(P)[None, :]
    tri = np.where(kk <= r, 0.0, NEG).astype(np.float32)
    if h == 0:
        mask_a, mask_b = tri, np.full((P, P), NEG, dtype=np.float32)
    else:
        mask_a, mask_b = np.zeros((P, P), dtype=np.float32), tri
    return mask_a.astype(ml_dtypes.bfloat16), mask_b.astype(ml_dtypes.bfloat16)


def _pow2_floor(x):
    return float(2.0 ** np.floor(np.log2(x)))


def kernel(
    inputs_for_keys,
    inputs_for_values,
    inputs_for_queries,
    weight_K,
    weight_V,
    weight_Q,
    trace=False,
):
    import ml_dtypes

    f8 = ml_dtypes.float8_e4m3

    xk_full = np.asarray(inputs_for_keys, dtype=np.float32)
    xv_full = np.asarray(inputs_for_values, dtype=np.float32)
    xq_full = np.asarray(inputs_for_queries, dtype=np.float32)
    w_v = np.asarray(weight_V, dtype=np.float32)
    w_qk = (
        np.asarray(weight_Q, dtype=np.float32)
        @ np.asarray(weight_K, dtype=np.float32).T
    )

    # power-of-two scales (range only; fp8 rel precision is scale-free)
    aq = _pow2_floor(192.0 / max(np.abs(xq_full).max(), 1e-30))
    ak = _pow2_floor(192.0 / max(np.abs(xk_full).max(), 1e-30))
    av = _pow2_floor(192.0 / max(np.abs(xv_full).max(), 1e-30))
    aqk = _pow2_floor(192.0 / max(np.abs(w_qk).max(), 1e-30))
    aw = _pow2_floor(192.0 / max(np.abs(w_v).max(), 1e-30))
    # G row scale: G = Xq@Wqk is near-Gaussian; absmax ~<= 8*sigma
    sg = float(
        np.sqrt((w_qk ** 2).mean() * D * (xq_full ** 2).mean())
    )
    ag = _pow2_floor(192.0 / (8.0 * sg))
    # U = attn@Xv (unnormalized): sigma_U^2 ~= sum_k E[a^2] * sigma_v^2
    ss = SCALE * sg * float(np.sqrt((xk_full ** 2).mean() * D))
    ea2 = float(np.exp(2.0 * ss * ss))
    su = float(np.sqrt(S * ea2 * (xv_full ** 2).mean()))
    au = _pow2_floor(16.0 / su)

    s_exp = SCALE / (ag * ak)
    g_qt = ag / (aq * aqk)      # psum(G*aq*aqk) -> qt = G*ag
    g_ut = au / av              # psum(U*av) -> ut = U*au
    c_ones = au * aw            # den vector value; rec = 1/(c*den)

    def hl(x, scale, layout):
        xs = np.asarray(x, np.float32) * scale
        hi = xs.astype(f8)
        lo = (xs - hi.astype(np.float32)).astype(f8)
        return layout(hi), layout(lo)

    def _xT(x):  # [rows, D] -> [P, DC, rows] (d on partitions)
        xt = np.asarray(x).T.reshape(DC, P, x.shape[0])
        return np.ascontiguousarray(xt.transpose(1, 0, 2))

    def _w(w):  # [D, D] -> [P, DC, D]
        wr = np.asarray(w).reshape(DC, P, D)
        return np.ascontiguousarray(wr.transpose(1, 0, 2))

    def _xv(x):  # [S, D] -> [P, S//P, D] (keys on partitions)
        xr = np.asarray(x).reshape(S // P, P, D)
        return np.ascontiguousarray(xr.transpose(1, 0, 2))

    wqh, wql = hl(w_qk, aqk, _w)
    wvh, wvl = hl(w_v, aw, _w)
    # pack wqk chunks [P, 8, 2, 2, 512]: chunk (a*4+dp) = hi/lo x row-pair
    # x column half a
    wq_pack = np.empty((P, 8, 2, 2, 512), dtype=f8)
    for a in range(2):
        for dp in range(DCP):
            cs = slice(a * 512, (a + 1) * 512)
            wq_pack[:, a * 4 + dp, 0] = wqh[:, 2 * dp:2 * dp + 2, cs]
            wq_pack[:, a * 4 + dp, 1] = wql[:, 2 * dp:2 * dp + 2, cs]
    xkT = [_xT((xk_full[b] * ak).astype(f8)) for b in range(B)]
    xvp = []
    for b in range(B):
        h_, l_ = hl(xv_full[b], av, _xv)
        xi = np.empty((P, S // P, 2, D), dtype=f8)
        xi[:, :, 0] = h_
        xi[:, :, 1] = l_
        xvp.append(xi)

    masks = [_make_masks(0), _make_masks(1)]
    ones_np = np.full((P, 1), c_ones, np.float32).astype(ml_dtypes.bfloat16)
    scl_np = np.zeros((P, 4), np.float32)
    scl_np[:, 0] = s_exp
    scl_np[:, 1] = g_qt
    scl_np[:, 2] = g_ut

    in_maps = []
    for c in range(2 * B):
        b, h = c // 2, c % 2
        rows = np.concatenate(
            [
                xq_full[b, 256 * s + 128 * h: 256 * s + 128 * h + P, :]
                for s in range(NSLOT)
            ],
            axis=0,
        )
        in_maps.append(
            {
                "xq": _xT((rows * aq).astype(f8)),
                "xk": xkT[b],
                "xv": xvp[b],
                "wq": wq_pack,
                "wvh": wvh,
                "wvl": wvl,
                "mask_a": masks[h][0],
                "mask_b": masks[h][1],
                "ones": ones_np,
                "scl": scl_np,
            }
        )

    nc = _get_nc()
    res = bass_utils.run_bass_kernel_spmd(
        nc, in_maps, core_ids=list(range(2 * B)), trace=trace
    )

    out = np.empty((B, S, D), dtype=np.float32)
    for c in range(2 * B):
        b, h = c // 2, c % 2
        o = np.asarray(res.results[c]["out"], dtype=np.float32)
        for s in range(NSLOT):
            out[b, 256 * s + 128 * h: 256 * s + 128 * h + P, :] = o[
                s * P:(s + 1) * P, :
            ]

    if trace:
        return out, res
    return out


# revision 3
# speedup vs baseline: 1.5139x; 1.0671x over previous
"""Causal single-head attention (B=4, S=2048, D=1024) on 8 TRN2 NeuronCores.

fp8(e4m3) DoubleRow rewrite of the bf16 baseline.  Same sharding: core
c -> (batch b = c//2, half h = c%2); 8 query slots of 128 rows with padded
causal key-lengths L_s = 256*(s+1); scores computed transposed; Wqk = Wq@Wk^T
fused host-side; out = ((attn @ X_v) @ W_v) / den.

Quantization scheme (all matmuls fp8e4 DoubleRow, PSUM fp32):
  G = Xq @ Wqk      2-product: Wqk shipped as exact hi+lo fp8 pair (host),
                    Xq single fp8.  G -> qt hi+lo on device (ACT copy + DVE
                    scalar_tensor_tensor residual).
  scores = qt@K^T   2-product: qt hi+lo (device), K^T single fp8 (host).
  U = attn @ Xv     3-product: attn hi+lo (ACT exp f8 + bf16, DVE sub),
                    Xv hi+lo (host).  U -> ut hi+lo on device.
  Y = U @ Wv        3-product: ut hi+lo (device), Wv hi+lo (host).
hi+lo fp8 pairs represent bf16 values exactly, so the residual GEMMs have
~bf16-level error; the 2-product GEMMs err only by the single side's fp8
quantization.  The softmax denominator is summed from the quantized bf16
attn (== hi+lo exactly), cancelling common-mode quantization error; the
"ones" den vector carries value au*aw so 1/den absorbs all output scales.
Scales are powers of two shipped in a [P,4] f32 vector, so the compiled
program is input-independent.

Schedule: the cost model serializes all DMA on one shared resource
(~360 GB/s) with ~0.6us per-issue overhead, so all bulk input DMA goes on
ONE queue (sync) in exact first-consumption order; consts + Xq ride the
scalar queue; output DMA on the SWDGE.  Slots run ASCENDING (0..7) so K/V
chunks stream just-in-time, and the 8 Y GEMMs are deferred to the end
(ut hi/lo tiles are tiny and stay resident) where they keep the PE busy
through the tail while Wv has long arrived.

NOTE (hazard, empirical): interleaving start/stop matmul accumulation chains
across sub-regions of a single PSUM bank corrupts results on HW; keep each
sub-tile's chain contiguous (interleaving across banks is fine).
"""

import numpy as np

import concourse.bacc as bacc
import concourse.mybir as mybir
import concourse.tile as tile
from concourse import bass_utils

B, S, D = 4, 2048, 1024
P = 128
DC = D // P          # 8 contraction chunks
DCP = DC // 2        # 4 DoubleRow contraction pairs
EC = D // P
NSLOT = 8
NQ = NSLOT * P
SCALE = 1.0 / float(np.sqrt(np.float32(S)))
NEG = -1.0e9

F32 = mybir.dt.float32
BF16 = mybir.dt.bfloat16
F8 = mybir.dt.float8e4
DR = mybir.MatmulPerfMode.DoubleRow
MULT = mybir.AluOpType.mult
SUB = mybir.AluOpType.subtract


def build_attention_nc():
    nc = bacc.Bacc("TRN2", target_bir_lowering=False)

    xq_in = nc.dram_tensor("xq", [P, DC, NQ], F8, kind="ExternalInput")
    xk_in = nc.dram_tensor("xk", [P, EC, S], F8, kind="ExternalInput")
    # X_v hi/lo interleaved per key tile: [p, tile, {hi,lo}, d]
    xv_in = nc.dram_tensor("xv", [P, S // P, 2, D], F8, kind="ExternalInput")
    # Wqk packed in consumption-ordered chunks: chunk c = (colhalf a, dcpair
    # dp) at c = a*4+dp, holding [{row0,row1}, 512 cols] (hi only; the G
    # GEMM runs single-fp8 -- Xq and Wqk each quantized once).  Chunk 0
    # rides in `boot` fused with the first Xq block so the first matmul
    # waits on a single DMA.
    wq_in = nc.dram_tensor("wq", [P, 8, 2, 512], F8, kind="ExternalInput")
    wvh_in = nc.dram_tensor("wvh", [P, DC, D], F8, kind="ExternalInput")
    wvl_in = nc.dram_tensor("wvl", [P, DC, D], F8, kind="ExternalInput")
    mask_a_in = nc.dram_tensor("mask_a", [P, P], BF16, kind="ExternalInput")
    mask_b_in = nc.dram_tensor("mask_b", [P, P], BF16, kind="ExternalInput")
    ones_in = nc.dram_tensor("ones", [P, 1], BF16, kind="ExternalInput")
    scl_in = nc.dram_tensor("scl", [P, 4], F32, kind="ExternalInput")
    out = nc.dram_tensor("out", [NQ, D], BF16, kind="ExternalOutput")

    with tile.TileContext(nc) as tc:
        with (
            tc.tile_pool(name="res", bufs=1) as res,
            tc.tile_pool(name="psum", bufs=4, space="PSUM") as pp,
        ):
            kt_sb = res.tile([P, EC, S], F8)        # K^T  [e, keys]
            xv_sb = res.tile([P, S // P, 2, D], F8)  # X_v hi/lo interleaved
            xq_sb = res.tile([P, DC, NQ], F8)       # X_q^T [d, q]
            qth_sb = res.tile([P, EC, NQ], F8)      # Q^T hi [e, q]
            qtl_sb = res.tile([P, EC, NQ], F8)      # Q^T lo
            wq_sb = res.tile([P, 8, 2, 512], F8)  # Wqk packed chunks
            wvh_sb = res.tile([P, DC, D], F8)
            wvl_sb = res.tile([P, DC, D], F8)
            uth_sb = res.tile([P, NSLOT, DC, P], F8)  # U^T hi (all slots)
            utl_sb = res.tile([P, NSLOT, DC, P], F8)
            rec_sb = res.tile([P, NSLOT], F32)        # 1/(c*den) per slot
            mask_a = res.tile([P, P], BF16)
            mask_b = res.tile([P, P], BF16)
            ones_sb = res.tile([P, 1], BF16)
            scl_sb = res.tile([P, 4], F32)
            s_exp = scl_sb[:, 0:1]   # exp scale = SCALE/(ag*ak)
            s_qt = scl_sb[:, 1:2]    # gamma_g: psum->qt scale
            s_ut = scl_sb[:, 2:3]    # gamma_u: psum->ut scale

            # ---- DMA: Xq-qb0 + consts on scalar; all bulk on sync, in
            # first-consumption order.  Each issue costs ~0.63us on its
            # queue, so the stream uses few, large chunks. ----
            nc.scalar.dma_start(xq_sb[:, 0:2, 0:512], xq_in[:, 0:2, 0:512])
            nc.gpsimd.dma_start(xq_sb[:, 2:8, 0:512], xq_in[:, 2:8, 0:512])
            nc.gpsimd.dma_start(scl_sb, scl_in[:, :])
            nc.gpsimd.dma_start(ones_sb, ones_in[:, :])
            nc.gpsimd.dma_start(mask_a, mask_a_in[:, :])
            nc.gpsimd.dma_start(mask_b, mask_b_in[:, :])
            nc.sync.dma_start(wq_sb[:, 0:1], wq_in[:, 0:1])
            nc.sync.dma_start(wq_sb[:, 1:4], wq_in[:, 1:4])
            nc.sync.dma_start(wq_sb[:, 4:8], wq_in[:, 4:8])
            nc.sync.dma_start(kt_sb[:, :, 0:512], xk_in[:, :, 0:512])
            nc.sync.dma_start(xv_sb[:, 0:2], xv_in[:, 0:2])
            nc.sync.dma_start(kt_sb[:, :, 512:1024], xk_in[:, :, 512:1024])
            nc.sync.dma_start(xv_sb[:, 2:4], xv_in[:, 2:4])
            nc.sync.dma_start(xq_sb[:, :, 512:NQ], xq_in[:, :, 512:NQ])
            nc.sync.dma_start(kt_sb[:, :, 1024:1536], xk_in[:, :, 1024:1536])
            nc.sync.dma_start(xv_sb[:, 4:8], xv_in[:, 4:8])
            nc.sync.dma_start(kt_sb[:, :, 1536:2048], xk_in[:, :, 1536:2048])
            nc.sync.dma_start(xv_sb[:, 8:12], xv_in[:, 8:12])
            nc.sync.dma_start(xv_sb[:, 12:16], xv_in[:, 12:16])
            nc.sync.dma_start(wvh_sb[:, :], wvh_in[:, :])
            nc.sync.dma_start(wvl_sb[:, :], wvl_in[:, :])

            def qt_copy(ec, cols, ps):
                nc.scalar.activation(
                    out=qth_sb[:, ec, cols], in_=ps,
                    func=mybir.ActivationFunctionType.Copy, scale=s_qt,
                )
                nc.vector.scalar_tensor_tensor(
                    out=qtl_sb[:, ec, cols], in0=ps, scalar=s_qt,
                    in1=qth_sb[:, ec, cols], op0=MULT, op1=SUB,
                )

            # ============ G^T phase ============
            # qb=0 (q cols 0:512 = slots 0-3, consumed first) dc-pair-outer
            # so compute starts as soon as the first wqk/xq chunks land.
            # wq_sb chunk c = (colhalf a)*4 + dp holds [{hi,lo}, pair, 512].
            for a in range(2):  # column half == ec group
                ps_list = [
                    pp.tile([P, 512], F32, tag="ps", name=f"psg{a}_{i}")
                    for i in range(4)
                ]
                for dp in range(DCP):
                    for i in range(4):
                        co = slice(i * P, (i + 1) * P)
                        nc.tensor.matmul(
                            ps_list[i], wq_sb[:, a * 4 + dp, :, co],
                            xq_sb[:, 2 * dp:2 * dp + 2, 0:512],
                            start=(dp == 0), stop=(dp == DCP - 1),
                            perf_mode=DR,
                        )
                for i in range(4):
                    qt_copy(a * 4 + i, slice(0, 512), ps_list[i])
            # qb=1 (q cols 512:1024) ec-outer; emitted lazily after slot 3
            # so the early slots aren't gated on the qb=1 Xq DMA
            def emit_gqb1(ec0, ec1):
                for ec in range(ec0, ec1):
                    ps = pp.tile([P, 512], F32, tag="ps")
                    a, i = ec // 4, ec % 4
                    co = slice(i * P, (i + 1) * P)
                    for dp in range(DCP):
                        nc.tensor.matmul(
                            ps, wq_sb[:, a * 4 + dp, :, co],
                            xq_sb[:, 2 * dp:2 * dp + 2, 512:NQ],
                            start=(dp == 0), stop=(dp == DCP - 1),
                            perf_mode=DR,
                        )
                    qt_copy(ec, slice(512, NQ), ps)

            # ================= attention =================
            with tc.tile_pool(name="attn", bufs=2) as ap:
                slot_state = {}
                pending = []

                def consume(s, g):
                    st = slot_state[s]
                    nt = st["nt"]
                    cnt = min(4, nt - g * 4)
                    a16, ah, al = st["a16"], st["ah"], st["al"]
                    for i in range(cnt):
                        t = g * 4 + i
                        nc.tensor.matmul(
                            st["ps_den"], a16[:, t, :], ones_sb,
                            start=(t == 0), stop=(t == nt - 1),
                        )
                    if g != st["ng"] - 1:
                        return
                    # slot finished: U^T hi/lo; Y deferred to the end
                    nc.vector.reciprocal(
                        rec_sb[:, s:s + 1], st["ps_den"]
                    )
                    np_ = nt // 2
                    for dh in range(2):
                        ps_u = pp.tile(
                            [P, 512], F32, tag="ua", bufs=2,
                            name=f"psu{s}_{dh}",
                        )
                        for i in range(4):
                            dc = dh * 4 + i
                            ds = slice(dc * P, (dc + 1) * P)
                            po = slice(i * P, (i + 1) * P)
                            for kp in range(np_):
                                ks = slice(2 * kp, 2 * kp + 2)
                                nc.tensor.matmul(
                                    ps_u[:, po], xv_sb[:, ks, 0, ds],
                                    ah[:, ks, :],
                                    start=(kp == 0), stop=False, perf_mode=DR,
                                )
                                nc.tensor.matmul(
                                    ps_u[:, po], xv_sb[:, ks, 0, ds],
                                    al[:, ks, :],
                                    start=False, stop=False, perf_mode=DR,
                                )
                                nc.tensor.matmul(
                                    ps_u[:, po], xv_sb[:, ks, 1, ds],
                                    ah[:, ks, :],
                                    start=False, stop=(kp == np_ - 1),
                                    perf_mode=DR,
                                )
                        ucols = slice(dh * 4, dh * 4 + 4)
                        nc.scalar.activation(
                            out=uth_sb[:, s, ucols, :], in_=ps_u,
                            func=mybir.ActivationFunctionType.Copy,
                            scale=s_ut,
                        )
                        nc.vector.scalar_tensor_tensor(
                            out=utl_sb[:, s, ucols, :], in0=ps_u,
                            scalar=s_ut, in1=uth_sb[:, s, ucols, :],
                            op0=MULT, op1=SUB,
                        )
                    del slot_state[s]

                for s in range(NSLOT):
                    if s == 4:
                        emit_gqb1(0, 8)
                        while pending:
                            consume(*pending.pop(0))
                    L = 256 * (s + 1)
                    nt = L // P
                    ng = (nt + 3) // 4
                    slot_state[s] = {
                        "nt": nt,
                        "ng": ng,
                        "a16": ap.tile(
                            [P, S // P, P], BF16, tag="a16", bufs=2,
                            name=f"a16_{s}",
                        ),
                        "ah": ap.tile(
                            [P, S // P, P], F8, tag="ah", bufs=2,
                            name=f"ah_{s}",
                        ),
                        "al": ap.tile(
                            [P, S // P, P], F8, tag="al", bufs=2,
                            name=f"al_{s}",
                        ),
                        "ps_den": pp.tile(
                            [P, 1], F32, tag="psden", bufs=2, name=f"psden{s}"
                        ),
                    }
                    st = slot_state[s]
                    qs = slice(s * P, (s + 1) * P)
                    for g in range(ng):
                        cnt = min(4, nt - g * 4)
                        psT = pp.tile([P, 512], F32, tag="ps")
                        for i in range(cnt):
                            t = g * 4 + i
                            po = slice(i * P, (i + 1) * P)
                            ts = slice(t * P, (t + 1) * P)
                            for j in range(DCP):
                                sl = slice(2 * j, 2 * j + 2)
                                nc.tensor.matmul(
                                    psT[:, po], kt_sb[:, sl, ts],
                                    qth_sb[:, sl, qs],
                                    start=(j == 0), stop=False, perf_mode=DR,
                                )
                                nc.tensor.matmul(
                                    psT[:, po], kt_sb[:, sl, ts],
                                    qtl_sb[:, sl, qs],
                                    start=False, stop=(j == DCP - 1),
                                    perf_mode=DR,
                                )
                        if g == ng - 1:
                            nc.vector.tensor_add(
                                out=psT[:, (cnt - 2) * P:(cnt - 1) * P],
                                in0=psT[:, (cnt - 2) * P:(cnt - 1) * P],
                                in1=mask_a,
                            )
                            nc.vector.tensor_add(
                                out=psT[:, (cnt - 1) * P:cnt * P],
                                in0=psT[:, (cnt - 1) * P:cnt * P],
                                in1=mask_b,
                            )
                        gs = slice(g * 4, g * 4 + cnt)
                        nc.scalar.activation(
                            out=st["a16"][:, gs, :], in_=psT[:, :cnt * P],
                            func=mybir.ActivationFunctionType.Exp,
                            scale=s_exp,
                        )
                        nc.scalar.activation(
                            out=st["ah"][:, gs, :], in_=psT[:, :cnt * P],
                            func=mybir.ActivationFunctionType.Exp,
                            scale=s_exp,
                        )
                        nc.vector.tensor_sub(
                            out=st["al"][:, gs, :],
                            in0=st["a16"][:, gs, :],
                            in1=st["ah"][:, gs, :],
                        )
                        if len(pending) >= 2:
                            consume(*pending.pop(0))
                        pending.append((s, g))
                while pending:
                    consume(*pending.pop(0))

                # ---- deferred Y = U @ Wv for all slots ----
                for s in range(NSLOT):
                    out_sb = ap.tile([P, D], BF16, tag="out", bufs=3)
                    rec = rec_sb[:, s:s + 1]
                    for eh in range(2):
                        es = slice(eh * 512, (eh + 1) * 512)
                        ps_y = pp.tile(
                            [P, 512], F32, tag="ps", bufs=4,
                            name=f"psy{eh}_{s}",
                        )
                        for dp in range(DCP):
                            sl = slice(2 * dp, 2 * dp + 2)
                            nc.tensor.matmul(
                                ps_y, uth_sb[:, s, sl, :], wvh_sb[:, sl, es],
                                start=(dp == 0), stop=False, perf_mode=DR,
                            )
                            nc.tensor.matmul(
                                ps_y, uth_sb[:, s, sl, :], wvl_sb[:, sl, es],
                                start=False, stop=False, perf_mode=DR,
                            )
                            nc.tensor.matmul(
                                ps_y, utl_sb[:, s, sl, :], wvh_sb[:, sl, es],
                                start=False, stop=(dp == DCP - 1),
                                perf_mode=DR,
                            )
                        for q in range(2):  # quarter-split for a short tail
                            qs_ = slice(eh * 512 + q * 256,
                                        eh * 512 + (q + 1) * 256)
                            if q == 0:
                                nc.scalar.activation(
                                    out=out_sb[:, qs_], in_=ps_y[:, 0:256],
                                    func=mybir.ActivationFunctionType.Copy,
                                    scale=rec,
                                )
                            else:
                                nc.vector.tensor_scalar_mul(
                                    out_sb[:, qs_], ps_y[:, 256:512], rec
                                )
                            eng = nc.gpsimd if (2 * eh + q) % 2 == 0 \
                                else nc.sync
                            eng.dma_start(
                                out[s * P:(s + 1) * P, qs_], out_sb[:, qs_]
                            )

    nc.compile()
    return nc


_NC_CACHE = None


def _get_nc():
    global _NC_CACHE
    if _NC_CACHE is None:
        _NC_CACHE = build_attention_nc()
    return _NC_CACHE


def _make_masks(h):
    """Transposed masks [key kk, query r] for the last two key tiles."""
    import ml_dtypes

    kk = np.arange(P)[:, None]
    r = np.arange(P)[None, :]
    tri = np.where(kk <= r, 0.0, NEG).astype(np.float32)
    if h == 0:
        mask_a, mask_b = tri, np.full((P, P), NEG, dtype=np.float32)
    else:
        mask_a, mask_b = np.zeros((P, P), dtype=np.float32), tri
    return mask_a.astype(ml_dtypes.bfloat16), mask_b.astype(ml_dtypes.bfloat16)


def _pow2_floor(x):
    return float(2.0 ** np.floor(np.log2(x)))


def kernel(
    inputs_for_keys,
    inputs_for_values,
    inputs_for_queries,
    weight_K,
    weight_V,
    weight_Q,
    trace=False,
):
    import ml_dtypes

    f8 = ml_dtypes.float8_e4m3

    xk_full = np.asarray(inputs_for_keys, dtype=np.float32)
    xv_full = np.asarray(inputs_for_values, dtype=np.float32)
    xq_full = np.asarray(inputs_for_queries, dtype=np.float32)
    w_v = np.asarray(weight_V, dtype=np.float32)
    w_qk = (
        np.asarray(weight_Q, dtype=np.float32)
        @ np.asarray(weight_K, dtype=np.float32).T
    )

    # power-of-two scales (range only; fp8 rel precision is scale-free)
    aq = _pow2_floor(192.0 / max(np.abs(xq_full).max(), 1e-30))
    ak = _pow2_floor(192.0 / max(np.abs(xk_full).max(), 1e-30))
    av = _pow2_floor(192.0 / max(np.abs(xv_full).max(), 1e-30))
    aqk = _pow2_floor(192.0 / max(np.abs(w_qk).max(), 1e-30))
    aw = _pow2_floor(192.0 / max(np.abs(w_v).max(), 1e-30))
    # G row scale: G = Xq@Wqk is near-Gaussian; absmax ~<= 8*sigma
    sg = float(
        np.sqrt((w_qk ** 2).mean() * D * (xq_full ** 2).mean())
    )
    ag = _pow2_floor(192.0 / (8.0 * sg))
    # U = attn@Xv (unnormalized): sigma_U^2 ~= sum_k E[a^2] * sigma_v^2
    ss = SCALE * sg * float(np.sqrt((xk_full ** 2).mean() * D))
    ea2 = float(np.exp(2.0 * ss * ss))
    su = float(np.sqrt(S * ea2 * (xv_full ** 2).mean()))
    au = _pow2_floor(16.0 / su)

    s_exp = SCALE / (ag * ak)
    g_qt = ag / (aq * aqk)      # psum(G*aq*aqk) -> qt = G*ag
    g_ut = au / av              # psum(U*av) -> ut = U*au
    c_ones = au * aw            # den vector value; rec = 1/(c*den)

    def hl(x, scale, layout):
        xs = np.asarray(x, np.float32) * scale
        hi = xs.astype(f8)
        lo = (xs - hi.astype(np.float32)).astype(f8)
        return layout(hi), layout(lo)

    def _xT(x):  # [rows, D] -> [P, DC, rows] (d on partitions)
        xt = np.asarray(x).T.reshape(DC, P, x.shape[0])
        return np.ascontiguousarray(xt.transpose(1, 0, 2))

    def _w(w):  # [D, D] -> [P, DC, D]
        wr = np.asarray(w).reshape(DC, P, D)
        return np.ascontiguousarray(wr.transpose(1, 0, 2))

    def _xv(x):  # [S, D] -> [P, S//P, D] (keys on partitions)
        xr = np.asarray(x).reshape(S // P, P, D)
        return np.ascontiguousarray(xr.transpose(1, 0, 2))

    wqh = _w((w_qk * aqk).astype(f8))
    wvh, wvl = hl(w_v, aw, _w)
    # pack wqk chunks [P, 8, 2, 512]: chunk (a*4+dp) = row-pair x col half a
    wq_pack = np.empty((P, 8, 2, 512), dtype=f8)
    for a in range(2):
        for dp in range(DCP):
            cs = slice(a * 512, (a + 1) * 512)
            wq_pack[:, a * 4 + dp] = wqh[:, 2 * dp:2 * dp + 2, cs]
    xkT = [_xT((xk_full[b] * ak).astype(f8)) for b in range(B)]
    xvp = []
    for b in range(B):
        h_, l_ = hl(xv_full[b], av, _xv)
        xi = np.empty((P, S // P, 2, D), dtype=f8)
        xi[:, :, 0] = h_
        xi[:, :, 1] = l_
        xvp.append(xi)

    masks = [_make_masks(0), _make_masks(1)]
    ones_np = np.full((P, 1), c_ones, np.float32).astype(ml_dtypes.bfloat16)
    scl_np = np.zeros((P, 4), np.float32)
    scl_np[:, 0] = s_exp
    scl_np[:, 1] = g_qt
    scl_np[:, 2] = g_ut

    in_maps = []
    for c in range(2 * B):
        b, h = c // 2, c % 2
        rows = np.concatenate(
            [
                xq_full[b, 256 * s + 128 * h: 256 * s + 128 * h + P, :]
                for s in range(NSLOT)
            ],
            axis=0,
        )
        xqc = _xT((rows * aq).astype(f8))
        in_maps.append(
            {
                "xq": xqc,
                "xk": xkT[b],
                "xv": xvp[b],
                "wq": wq_pack,
                "wvh": wvh,
                "wvl": wvl,
                "mask_a": masks[h][0],
                "mask_b": masks[h][1],
                "ones": ones_np,
                "scl": scl_np,
            }
        )

    nc = _get_nc()
    res = bass_utils.run_bass_kernel_spmd(
        nc, in_maps, core_ids=list(range(2 * B)), trace=trace
    )

    out = np.empty((B, S, D), dtype=np.float32)
    for c in range(2 * B):
        b, h = c // 2, c % 2
        o = np.asarray(res.results[c]["out"], dtype=np.float32)
        for s in range(NSLOT):
            out[b, 256 * s + 128 * h: 256 * s + 128 * h + P, :] = o[
                s * P:(s + 1) * P, :
            ]

    if trace:
        return out, res
    return out
